# revision 33
# baseline (speedup 1.0000x reference)
"""Trainium2 Bass kernel for modulated conv1d (StyleGAN-style Conv1DMod).

Reference computation (per batch sample b):
  wm[k,c,f]  = kern[k,c,f] * coef * (style[b,c] + 1)        (modulate)
  denom[f]   = rsqrt(sum_{k,c} wm[k,c,f]^2)                 (demodulate)
  out[b,f,w] = denom[f] * sum_{k,c} wm[k,c,f] * feat[b,c,w+k-1]   (SAME conv)

Sharding: data-parallel over batch B=8 -> one sample per NeuronCore.

Design notes (from iterative trace analysis; baseline fp32r kernel ran
71.9us, this one ~61us; the floor for this decomposition is ~59.5us:
~13.5us NEFF-preamble + minimum-feed before the matmul stream can start,
41.5us of PE streaming at the 216 ns/matmul issue floor, ~4.3us of
copy/store/receipt/teardown tail):
  - The NEFF preamble (engine barriers + register init) means no PE
    instruction before ~7.9us and no DMA bytes before ~9.3us; the first
    ~1.3MB of input (weight taps + first feature pieces) then lands at a
    shared ~350-430 GB/s, so real matmuls cannot start before ~13us.
    Junk warm-up matmuls from ~7.9us flip the PE HAM clock-gate to
    8/8 (2.4 GHz) before the real stream starts, and junk 128-col
    "filler" matmuls pad the arrival-paced ramp so the PE never idles
    long enough for the (retroactive, ~3.4us window) HAM re-throttle.
  - Everything on the PE runs in bf16 (tolerance is 2e-2; this path
    measures 3.6e-3): weights are modulated into bf16, features are
    DMA-cast fp32->bf16 by the SWDGE (gpsimd) queue at line rate (the
    cast was measured free at >=2KB descriptor runs), and FWL makes the
    per-matmul LDWEIGHTS fully hidden (fp32r paid ~45ns/MM extra).
  - The gpsimd SWDGE ring carries ONLY the feature stream, in exact
    consumption order (chunk 0 in 513-col quarters for a fast head).
    Weight taps ride the two HWDGE queues (k0 on sync behind the tiny
    style DMA, k1+k2 on scalar) and are modulated per (tap, ct-half) as
    they land (vector=ct0, scalar=ct1; tap k0 entirely on vector, and a
    dummy activation pre-triggers the scalar engine's lazy 1.28us
    ACT_TABLE_LOAD); accumulation groups run k-major
    so the first group starts on tap-0 only. Cross-queue packet
    round-robin starves small transfers behind a firehose (measured
    +8us on a weight tap), so the early window keeps every queue small
    except the feature stream.
  - Demodulation is a per-(b,f) scale applied during the PSUM->SBUF
    copies (output partition dim is f), which also downconvert to bf16
    so output stores move half the bytes. Copies alternate ACT/DVE;
    denom comes from DVE squares/sums + 4 tiny PE dot-products slotted
    into the stream behind the first conv groups.
  - Stores are per-512-col pieces on sync (a 1MB store burst at a chunk
    boundary was measured to starve the input stream), and the final
    group is split into two 256-col groups on separate PSUM tiles with
    the last store on the scalar queue, shortening the end-of-kernel
    receipt tail.
"""

import numpy as np

import concourse.bass as bass
import concourse.mybir as mybir
import concourse.tile as tile

B, C, W, K, F = 8, 256, 8192, 3, 256
COEF = 1.0 / float(np.sqrt(K * C))

P = 128
CT = C // P  # 2 contraction tiles
FT = F // P  # 2 output-partition tiles
WCHUNK = 2048  # output store granularity
NJ = W // WCHUNK  # 4 chunks
WTILE = 512  # matmul moving-operand width (= one PSUM bank of fp32)
NI = WCHUNK // WTILE  # 4 w-tiles per chunk
XCOLS = WCHUNK + 2  # chunk + 1-col halo each side

N_WARM = 9  # 512-col junk matmuls; a fine 128-col tail follows for slot-in granularity
N_WARM_FINE = 7

MAX_WAITS = 1  # walrus codegen in this container rejects >1 sync wait per inst


def _split_sync_waits(nc, limit=MAX_WAITS):
    """Move excess sem-waits onto NoOps inserted before the offending
    instruction (same engine, program order preserved)."""
    uid = 0
    for fn in nc.m.functions:
        for bb in fn.blocks:
            insts = bb.instructions
            changed = False
            newlist = []
            for ins in insts:
                si = ins.sync_info
                if si is not None and len(si.on_wait) > limit:
                    waits = list(si.on_wait)
                    keep = waits[-limit:]
                    excess = waits[:-limit]
                    for k in range(0, len(excess), limit):
                        nop = mybir.InstNoOp(name=f"waitsplit-{uid}", ins=[], outs=[])
                        uid += 1
                        nop.engine = ins.engine
                        nop.sync_info = mybir.SyncInfo(
                            on_wait=excess[k : k + limit], on_update=[]
                        )
                        newlist.append(nop)
                    ins.sync_info = mybir.SyncInfo(
                        on_wait=keep, on_update=list(si.on_update)
                    )
                    changed = True
                newlist.append(ins)
            if changed:
                bb.instructions = newlist


def _conv1dmod_body(tc, feat, style, kern, out):
    nc = tc.nc
    f32 = mybir.dt.float32
    bf16 = mybir.dt.bfloat16

    with (
        tc.tile_pool(name="xbuf", bufs=1) as xbuf,
        tc.tile_pool(name="wbuf", bufs=1) as wbuf,
        tc.tile_pool(name="stage", bufs=1) as stage_pool,
        tc.tile_pool(name="psum", bufs=6, space="PSUM") as psum_pool,
        tc.tile_pool(name="dpsum", bufs=1, space="PSUM") as dpsum_pool,
    ):
        # ---- PE warm-up: junk bf16 matmuls from the moment the engines come
        # out of the preamble until the first real weights/features land.
        # This flips the HAM clock-gate to 8/8 (2.4 GHz) before real work.
        junk_w = wbuf.tile([P, P], bf16, tag="junk_w", name="junk_w")
        junk_x = wbuf.tile([P, WTILE], bf16, tag="junk_x", name="junk_x")
        nc.gpsimd.memset(junk_w[:], 0.0)
        nc.gpsimd.memset(junk_x[:], 0.0)

        # ---- All input streams ride ONE queue (gpsimd SWDGE) in exact
        # consumption order: weight taps 0-1, first feature pieces, tap 2,
        # rest of chunk 0, chunks 1-3. A strict-FIFO single ring gets the
        # full SDMA rate and cannot starve the weight taps the way
        # cross-queue packet round-robin was measured to (v3: tap-2 landed
        # at 19.9us behind the feature firehose on another queue).
        # Only style (tiny, sync) and output stores (sync) use other queues.
        ssty = wbuf.tile([P, CT], f32, tag="ssty", name="ssty")
        with nc.allow_non_contiguous_dma(reason="256-elem style vector"):
            nc.sync.dma_start(ssty[:], style.rearrange("(o p) -> p o", p=P))

        xt = [[None] * NJ for _ in range(CT)]
        for ct in range(CT):
            for j in range(NJ):
                xt[ct][j] = xbuf.tile([P, XCOLS], bf16, tag=f"x_{ct}_{j}", name=f"x_{ct}_{j}")
        for ct in range(CT):
            nc.vector.memset(xt[ct][0][:, 0:1], 0.0)
            nc.vector.memset(xt[ct][NJ - 1][:, XCOLS - 1 : XCOLS], 0.0)

        def emit_load(j, ct, c0, c1):
            """DMA tile cols [c0, c1) of chunk j; tile col 0 = w (j*2048 - 1)."""
            w0 = j * WCHUNK - 1 + c0
            w1 = j * WCHUNK - 1 + c1
            crow = slice(ct * P, (ct + 1) * P)
            nc.gpsimd.dma_start(xt[ct][j][:, c0:c1], feat[crow, w0:w1])

        # Weight taps ride the two HWDGE queues (sync gets k0 right after the
        # tiny style DMA, scalar gets k1+k2); the gpsimd SWDGE ring is a pure
        # feature stream. All three queues are small-and-early except the
        # feature firehose, so nothing gets starved by packet round-robin.
        ksb = wbuf.tile([P, K, CT, F], f32, tag="ksb", name="ksb")
        nc.sync.dma_start(ksb[:, 0], kern[0].rearrange("(h p) f -> p h f", p=P))
        nc.scalar.dma_start(ksb[:, 1], kern[1].rearrange("(h p) f -> p h f", p=P))
        nc.scalar.dma_start(ksb[:, 2], kern[2].rearrange("(h p) f -> p h f", p=P))
        # dummy activation: pulls the scalar engine's lazy ACT_TABLE_LOAD
        # (1.28us) into the idle window before any real ACTIVATE needs it
        dummy = wbuf.tile([P, 1], f32, tag="dummy", name="dummy")
        nc.scalar.mul(dummy[:], junk_w[:, 0:1], 1.0)
        # chunk 0 in quarter pieces, ct-interleaved
        J0_BOUNDS = [1, 514, 1026, 1538, XCOLS]
        for p0, p1 in zip(J0_BOUNDS[:-1], J0_BOUNDS[1:]):
            for ct in range(CT):
                emit_load(0, ct, p0, p1)
        for ct in range(CT):
            emit_load(1, ct, 0, 1027)
        for ct in range(CT):
            emit_load(1, ct, 1027, XCOLS)
        for j in range(2, NJ):
            for ct in range(CT):
                if j == NJ - 1:
                    emit_load(j, ct, 0, XCOLS - 1)
                else:
                    emit_load(j, ct, 0, XCOLS)

        # ---- warm-up matmuls (junk data, dead PSUM bank)
        warm_ps = psum_pool.tile([P, WTILE], f32, tag="warm", bufs=1, name="warm_ps")
        for _ in range(N_WARM):
            nc.tensor.matmul(warm_ps[:], junk_w[:], junk_x[:], start=True, stop=True)
        for _ in range(N_WARM_FINE):
            nc.tensor.matmul(
                warm_ps[:, :P], junk_w[:], junk_x[:, :P], start=True, stop=True
            )

        # ---- modulate: wm[c, ct, k, f] = ksb * coef * (style+1), bf16 out.
        # Split per tap across vector (ct0) and scalar (ct1) so each tap's
        # weights are ready ~0.5us after its DMA lands.
        s1 = wbuf.tile([P, CT], f32, tag="s1", name="s1")
        nc.vector.tensor_scalar(
            s1[:], ssty[:], 1.0, COEF, mybir.AluOpType.add, mybir.AluOpType.mult
        )
        wm = wbuf.tile([P, CT, K, F], bf16, tag="wm", name="wm")
        nc.vector.tensor_scalar_mul(wm[:, 0, 0], ksb[:, 0, 0], s1[:, 0:1])
        nc.vector.tensor_scalar_mul(wm[:, 1, 0], ksb[:, 0, 1], s1[:, 1:2])
        for k in range(1, K):
            nc.vector.tensor_scalar_mul(wm[:, 0, k], ksb[:, k, 0], s1[:, 0:1])
            nc.scalar.mul(wm[:, 1, k], ksb[:, k, 1], s1[:, 1:2])

        def emit_group(j, ft, i, fillers=0, tail_fillers=0):
            """One PSUM accumulation group (k-major so it can start on tap-0
            weights); `fillers` small junk matmuls after each k-pair keep the
            PE busy/warm while the next tap's weights are still in flight."""
            ps = psum_pool.tile([P, WTILE], f32, tag="ps", name="ps")
            n = 0
            for k in range(K):
                for ct in range(CT):
                    n += 1
                    nc.tensor.matmul(
                        ps[:],
                        wm[:, ct, k, ft * P : (ft + 1) * P],
                        xt[ct][j][:, i * WTILE + k : i * WTILE + k + WTILE],
                        start=(n == 1),
                        stop=(n == K * CT),
                    )
                if fillers and k < K - 1:
                    emit_fillers(fillers)
            emit_fillers(tail_fillers)
            return ps

        def emit_fillers(n):
            for _ in range(n):
                nc.tensor.matmul(
                    warm_ps[:, :P], junk_w[:], junk_x[:, :P],
                    start=True, stop=True,
                )

        def emit_mms(j, ft):
            return [emit_group(j, ft, i) for i in range(NI)]

        def emit_copies(j, ft, pss):
            """Demodulating PSUM->SBUF bf16 copies (alternate DVE/ACT) and
            output stores."""
            st = stage_pool.tile([P, WCHUNK], bf16, tag=f"st_{j}_{ft}", name=f"st_{j}_{ft}")
            for i, ps in enumerate(pss):
                dst = st[:, i * WTILE : (i + 1) * WTILE]
                if i % 2 == 0:
                    nc.scalar.mul(dst, ps[:], denom[:, ft : ft + 1])
                else:
                    nc.vector.tensor_scalar_mul(dst, ps[:], denom[:, ft : ft + 1])
            out_rows = slice(ft * P, (ft + 1) * P)
            # per-i pieces: smooths the HBM write traffic (no 1MB bursts
            # starving the input stream at chunk boundaries) and lets the
            # final store trail the last copy by as little as possible
            npieces = NI
            piece = WCHUNK // npieces
            for h in range(npieces):
                out_cols = slice(j * WCHUNK + h * piece, j * WCHUNK + (h + 1) * piece)
                nc.sync.dma_start(
                    out[out_rows, out_cols], st[:, h * piece : (h + 1) * piece]
                )

        # first conv groups go ahead of the demod chain on the PE queue;
        # group 0 carries ramp fillers (weight taps are still landing)
        pss00 = [emit_group(0, 0, 0, fillers=4, tail_fillers=4)]
        pss00.append(emit_group(0, 0, 1, fillers=1, tail_fillers=3))
        pss00.append(emit_group(0, 0, 2, tail_fillers=2))

        # ---- demodulation scale: denom[f] = rsqrt(sum_{k,c} wm^2).
        # Squares/sums on DVE in fp32, column-sum via 4 tiny PE dots (queued
        # behind the first conv groups), sqrt on ACT, reciprocal on DVE.
        sq = wbuf.tile([P, CT, K, F], f32, tag="sq", name="sq")
        sqt = wbuf.tile([P, F], f32, tag="sqt", name="sqt")
        ssq = wbuf.tile([P, CT, F], bf16, tag="ssq", name="ssq")
        for ct in range(CT):
            nc.vector.tensor_mul(sq[:, ct], wm[:, ct], wm[:, ct])
            nc.vector.tensor_add(sqt[:], sq[:, ct, 0], sq[:, ct, 1])
            nc.vector.tensor_add(ssq[:, ct], sqt[:], sq[:, ct, 2])
        ones = wbuf.tile([P, 1], bf16, tag="ones", name="ones")
        nc.vector.memset(ones[:], 1.0)
        dp = dpsum_pool.tile([P, FT], f32, tag="dp", name="dp")
        for ft in range(FT):
            for ct in range(CT):
                nc.tensor.matmul(
                    dp[:, ft : ft + 1],
                    ssq[:, ct, ft * P : (ft + 1) * P],
                    ones[:],
                    start=(ct == 0),
                    stop=(ct == CT - 1),
                )
        denom = wbuf.tile([P, FT], f32, tag="denom", name="denom")
        nc.scalar.activation(denom[:], dp[:], mybir.ActivationFunctionType.Sqrt)
        nc.vector.reciprocal(denom[:], denom[:])

        # ---- main conv stream
        pss00.append(emit_group(0, 0, 3))
        emit_copies(0, 0, pss00)
        emit_copies(0, 1, emit_mms(0, 1))
        for j in range(1, NJ):
            for ft in range(FT):
                if j == NJ - 1 and ft == FT - 1:
                    break
                emit_copies(j, ft, emit_mms(j, ft))

        # last (chunk, ft): final accumulation group split into two 256-col
        # groups on separate PSUM tiles so the very last copy+store are
        # half-size (shorter end-of-kernel tail)
        j, ft = NJ - 1, FT - 1
        pss = [emit_group(j, ft, i) for i in range(NI - 1)]
        st = stage_pool.tile([P, WCHUNK], bf16, tag="st_last", name="st_last")
        for i, ps in enumerate(pss):
            dst = st[:, i * WTILE : (i + 1) * WTILE]
            if i % 2 == 0:
                nc.scalar.mul(dst, ps[:], denom[:, ft : ft + 1])
            else:
                nc.vector.tensor_scalar_mul(dst, ps[:], denom[:, ft : ft + 1])
        out_rows = slice(ft * P, (ft + 1) * P)
        for h in range(NI - 1):
            oc = slice(j * WCHUNK + h * WTILE, j * WCHUNK + (h + 1) * WTILE)
            nc.sync.dma_start(out[out_rows, oc], st[:, h * WTILE : (h + 1) * WTILE])
        i = NI - 1
        HW = WTILE // 2
        for half in range(2):
            psl = psum_pool.tile([P, WTILE], f32, tag="ps", name="psl")
            h0 = half * HW
            n = 0
            for k in range(K):
                for ct in range(CT):
                    n += 1
                    base = i * WTILE + h0 + k
                    nc.tensor.matmul(
                        psl[:, :HW],
                        wm[:, ct, k, ft * P : (ft + 1) * P],
                        xt[ct][j][:, base : base + HW],
                        start=(n == 1),
                        stop=(n == K * CT),
                    )
            if half == 0:
                dst = st[:, i * WTILE + h0 : i * WTILE + h0 + HW]
                nc.scalar.mul(dst, psl[:, :HW], denom[:, ft : ft + 1])
                oc = slice(
                    j * WCHUNK + i * WTILE + h0, j * WCHUNK + i * WTILE + h0 + HW
                )
                nc.sync.dma_start(out[out_rows, oc], dst)
            else:
                # very last 256 cols: quarter-width copies on both engines
                # with parallel quarter-stores on both queues, so the final
                # HBM write receipt trails the last matmul minimally
                QW = HW // 2
                for q in range(2):
                    c0 = i * WTILE + h0 + q * QW
                    dst = st[:, c0 : c0 + QW]
                    psrc = psl[:, q * QW : (q + 1) * QW]
                    if q == 0:
                        nc.vector.tensor_scalar_mul(dst, psrc, denom[:, ft : ft + 1])
                    else:
                        nc.scalar.mul(dst, psrc, denom[:, ft : ft + 1])
                    oc = slice(j * WCHUNK + c0, j * WCHUNK + c0 + QW)
                    deng = nc.sync if q == 0 else nc.scalar
                    deng.dma_start(out[out_rows, oc], dst)


def build_bass():
    nc = bass.Bass(name="conv1dmod")
    feat = nc.dram_tensor("feature", [C, W], mybir.dt.float32, kind="ExternalInput")
    style = nc.dram_tensor("style", [C], mybir.dt.float32, kind="ExternalInput")
    kern = nc.dram_tensor("kern", [K, C, F], mybir.dt.float32, kind="ExternalInput")
    out = nc.dram_tensor("out", [F, W], mybir.dt.bfloat16, kind="ExternalOutput")
    with tile.TileContext(nc) as tc:
        _conv1dmod_body(tc, feat, style, kern, out)
    _split_sync_waits(nc)
    return nc


_NC_CACHE = None


def kernel(feature, style, kernel):
    """Full-input entry point: shard over batch across 8 cores, run, gather."""
    global _NC_CACHE
    from concourse.bass_utils import run_bass_kernel_spmd

    if _NC_CACHE is None:
        _NC_CACHE = build_bass()
    nc = _NC_CACHE

    feature = np.ascontiguousarray(feature, dtype=np.float32)
    style = np.ascontiguousarray(style, dtype=np.float32)
    kernel = np.ascontiguousarray(kernel, dtype=np.float32)

    in_maps = [
        {"feature": feature[b], "style": style[b], "kern": kernel} for b in range(B)
    ]
    res = run_bass_kernel_spmd(nc, in_maps, core_ids=list(range(B)))
    return np.stack(
        [np.asarray(r["out"]).astype(np.float32) for r in res.results], axis=0
    )
